# revision 2
# baseline (speedup 1.0000x reference)
"""DenseGeneralAqt inference kernel for Trainium2 (8 NeuronCores).

out = (x @ dequant_int8(qkernel)) * qscale,  x:(2,2048,1024) f32,
qkernel:(1024,4096) int8, qscale:(1,4096) f32 -> out:(2,2048,4096) f32.

Strategy: data-parallel over the flattened token axis (4096 rows -> 512
rows/core). Each core keeps the full weight in SBUF as bf16 (int8 values
are exact in bf16), casts its x shard to bf16, transposes it via the DMA
xbar so K lands on partitions, runs 8-deep accumulating bf16 matmuls per
(m, n) output tile, and fuses the per-channel scale into the PSUM->SBUF
drain on the vector engine.
"""

import numpy as np

P = 128
B, S, D, F = 2, 2048, 1024, 4096
N_CORES = 8
M_FULL = B * S                    # 4096 rows
M_CORE = M_FULL // N_CORES        # 512 rows per core
NT = 512                          # n-tile (one PSUM bank of f32)
WM, WK, WN = M_CORE // P, D // P, F // NT

_CACHE: dict = {}


def _build():
    import concourse.tile as tile
    from concourse import bacc, mybir

    nc = bacc.Bacc("TRN2", target_bir_lowering=False, debug=False)

    x_dram = nc.dram_tensor("x", [M_CORE, D], mybir.dt.float32, kind="ExternalInput")
    w_dram = nc.dram_tensor("w", [D, F], mybir.dt.int8, kind="ExternalInput")
    s_dram = nc.dram_tensor("s", [1, F], mybir.dt.float32, kind="ExternalInput")
    o_dram = nc.dram_tensor("o", [M_CORE, F], mybir.dt.float32, kind="ExternalOutput")

    with tile.TileContext(nc) as tc:
        with (
            tc.tile_pool(name="w", bufs=1) as wp,
            tc.tile_pool(name="qs", bufs=1) as qp,
            tc.tile_pool(name="xf", bufs=2) as xfp,
            tc.tile_pool(name="xb", bufs=2) as xbp,
            tc.tile_pool(name="xt", bufs=2) as xtp,
            tc.tile_pool(name="o", bufs=4) as op,
            tc.tile_pool(name="ps", bufs=4, space="PSUM") as pp,
        ):
            # Full weight, cast int8->bf16 in the DMA datapath (SWDGE).
            w_sb = [
                wp.tile([P, F], mybir.dt.bfloat16, name=f"w{kt}", tag=f"w{kt}")
                for kt in range(WK)
            ]
            for kt in range(WK):
                nc.gpsimd.dma_start(w_sb[kt][:], w_dram[kt * P:(kt + 1) * P, :])

            # Per-channel scale replicated across partitions.
            qs = qp.tile([P, F], mybir.dt.float32)
            nc.gpsimd.dma_start(qs[:], s_dram[0:1, :].to_broadcast((P, F)))

            for mt in range(WM):
                xf = xfp.tile([P, D], mybir.dt.float32)
                nc.sync.dma_start(xf[:], x_dram[mt * P:(mt + 1) * P, :])
                xb = xbp.tile([P, D], mybir.dt.bfloat16)
                nc.vector.tensor_copy(xb[:], xf[:])
                xt = xtp.tile([P, WK, P], mybir.dt.bfloat16)
                for kt in range(WK):
                    nc.sync.dma_start(
                        xt[:, kt, :], xb[:, kt * P:(kt + 1) * P], transpose=True
                    )
                for nt in range(WN):
                    ps = pp.tile([P, NT], mybir.dt.float32)
                    for kt in range(WK):
                        nc.tensor.matmul(
                            ps[:],
                            xt[:, kt, :],
                            w_sb[kt][:, nt * NT:(nt + 1) * NT],
                            start=(kt == 0),
                            stop=(kt == WK - 1),
                        )
                    ot = op.tile([P, NT], mybir.dt.float32)
                    nc.vector.tensor_mul(ot[:], ps[:], qs[:, nt * NT:(nt + 1) * NT])
                    nc.sync.dma_start(
                        o_dram[mt * P:(mt + 1) * P, nt * NT:(nt + 1) * NT], ot[:]
                    )

    nc.compile()
    return nc


def _get_nc():
    if "nc" not in _CACHE:
        _CACHE["nc"] = _build()
    return _CACHE["nc"]


def _run(x, qkernel, qscale, trace=False):
    from concourse.bass_utils import run_bass_kernel_spmd

    x = np.ascontiguousarray(np.asarray(x), dtype=np.float32).reshape(M_FULL, D)
    w = np.asarray(qkernel)
    if w.dtype != np.int8:
        w = w.astype(np.int8)
    w = np.ascontiguousarray(w)
    s = np.ascontiguousarray(np.asarray(qscale), dtype=np.float32).reshape(1, F)

    in_maps = [
        {"x": x[c * M_CORE:(c + 1) * M_CORE], "w": w, "s": s} for c in range(N_CORES)
    ]
    res = run_bass_kernel_spmd(
        _get_nc(), in_maps, core_ids=list(range(N_CORES)), trace=trace
    )
    out = np.concatenate([res.results[c]["o"] for c in range(N_CORES)], axis=0)
    return out.reshape(B, S, F), res


def kernel(x, qkernel, qscale):
    out, _ = _run(x, qkernel, qscale, trace=False)
    return out


def kernel_traced(x, qkernel, qscale):
    out, res = _run(x, qkernel, qscale, trace=True)
    return out, res


# revision 5
# speedup vs baseline: 1.0991x; 1.0991x over previous
"""DenseGeneralAqt inference kernel for Trainium2 (8 NeuronCores).

out = (x @ dequant_int8(qkernel)) * qscale,  x:(2,2048,1024) f32,
qkernel:(1024,4096) int8, qscale:(1,4096) f32 -> out:(2,2048,4096) f32.

Strategy: data-parallel over the flattened token axis (4096 rows -> 512
rows/core). Each core keeps the full weight in SBUF as fp16 (int8 values
are exact in fp16), cast in the SWDGE DMA datapath. Its x shard is cast
to fp16 and transposed via one batched DMA-xbar transpose per m-tile so
K lands on partitions. The matmul loop is k-outer over all 8 PSUM banks
so the PE starts as soon as weight k-tile 0 arrives, and the per-channel
scale is fused into the PSUM->SBUF drain on the vector engine. Output
DMAs ride the Scalar (ACT) HWDGE queue to keep the Sync queue free for
x loads and transposes.
"""

import numpy as np

P = 128
B, S, D, F = 2, 2048, 1024, 4096
N_CORES = 8
M_FULL = B * S                    # 4096 rows
M_CORE = M_FULL // N_CORES        # 512 rows per core
NT = 512                          # n-tile (one PSUM bank of f32)
WM, WK, WN = M_CORE // P, D // P, F // NT

_CACHE: dict = {}


def _build():
    import concourse.tile as tile
    from concourse import bacc, mybir

    nc = bacc.Bacc("TRN2", target_bir_lowering=False, debug=False)

    x_dram = nc.dram_tensor("x", [M_CORE, D], mybir.dt.float32, kind="ExternalInput")
    w_dram = nc.dram_tensor("w", [D, F], mybir.dt.int8, kind="ExternalInput")
    s_dram = nc.dram_tensor("s", [1, F], mybir.dt.float32, kind="ExternalInput")
    o_dram = nc.dram_tensor("o", [M_CORE, F], mybir.dt.float32, kind="ExternalOutput")

    with tile.TileContext(nc) as tc:
        with (
            tc.tile_pool(name="w", bufs=1) as wp,
            tc.tile_pool(name="qs", bufs=1) as qp,
            tc.tile_pool(name="xf", bufs=2) as xfp,
            tc.tile_pool(name="xb", bufs=2) as xbp,
            tc.tile_pool(name="xt", bufs=2) as xtp,
            tc.tile_pool(name="o", bufs=6) as op,
            tc.tile_pool(name="ps", bufs=8, space="PSUM") as pp,
        ):
            # Full weight, cast int8->fp16 in the DMA datapath (SWDGE).
            w_sb = [
                wp.tile([P, F], mybir.dt.float16, name=f"w{kt}", tag=f"w{kt}")
                for kt in range(WK)
            ]
            for kt in range(WK):
                nc.gpsimd.dma_start(w_sb[kt][:], w_dram[kt * P:(kt + 1) * P, :])

            # Per-channel scale replicated across partitions (HWDGE on ACT).
            qs = qp.tile([P, F], mybir.dt.float32)
            nc.scalar.dma_start(qs[:], s_dram[0:1, :].to_broadcast((P, F)))

            for mt in range(WM):
                xf = xfp.tile([P, D], mybir.dt.float32)
                nc.sync.dma_start(xf[:], x_dram[mt * P:(mt + 1) * P, :])
                xb = xbp.tile([P, D], mybir.dt.float16)
                nc.vector.tensor_copy(xb[:], xf[:])
                # One xbar transpose per m-tile: [128m, 1024k] -> [128k, 8kt, 128m]
                xt = xtp.tile([P, WK, P], mybir.dt.float16)
                nc.sync.dma_start(xt[:, :, :], xb[:, :], transpose=True)

                ps = [
                    pp.tile([P, NT], mybir.dt.float32, name=f"ps{mt}_{nt}", tag="ps")
                    for nt in range(WN)
                ]
                for kt in range(WK):
                    for nt in range(WN):
                        nc.tensor.matmul(
                            ps[nt][:],
                            xt[:, kt, :],
                            w_sb[kt][:, nt * NT:(nt + 1) * NT],
                            start=(kt == 0),
                            stop=(kt == WK - 1),
                        )
                for nt in range(WN):
                    ot = op.tile([P, NT], mybir.dt.float32, name=f"o{mt}_{nt}", tag="o")
                    nc.vector.tensor_mul(ot[:], ps[nt][:], qs[:, nt * NT:(nt + 1) * NT])
                    nc.scalar.dma_start(
                        o_dram[mt * P:(mt + 1) * P, nt * NT:(nt + 1) * NT], ot[:]
                    )

    nc.compile()
    return nc


def _get_nc():
    if "nc" not in _CACHE:
        _CACHE["nc"] = _build()
    return _CACHE["nc"]


def _run(x, qkernel, qscale, trace=False):
    from concourse.bass_utils import run_bass_kernel_spmd

    x = np.ascontiguousarray(np.asarray(x), dtype=np.float32).reshape(M_FULL, D)
    w = np.asarray(qkernel)
    if w.dtype != np.int8:
        w = w.astype(np.int8)
    w = np.ascontiguousarray(w)
    s = np.ascontiguousarray(np.asarray(qscale), dtype=np.float32).reshape(1, F)

    in_maps = [
        {"x": x[c * M_CORE:(c + 1) * M_CORE], "w": w, "s": s} for c in range(N_CORES)
    ]
    res = run_bass_kernel_spmd(
        _get_nc(), in_maps, core_ids=list(range(N_CORES)), trace=trace
    )
    out = np.concatenate([res.results[c]["o"] for c in range(N_CORES)], axis=0)
    return out.reshape(B, S, F), res


def kernel(x, qkernel, qscale):
    out, _ = _run(x, qkernel, qscale, trace=False)
    return out


def kernel_traced(x, qkernel, qscale):
    out, res = _run(x, qkernel, qscale, trace=True)
    return out, res


# revision 6
# speedup vs baseline: 1.2497x; 1.1369x over previous
"""DenseGeneralAqt inference kernel for Trainium2 (8 NeuronCores).

out = (x @ dequant_int8(qkernel)) * qscale,  x:(2,2048,1024) f32,
qkernel:(1024,4096) int8, qscale:(1,4096) f32 -> out:(2,2048,4096) f32.

Strategy: data-parallel over the flattened token axis (4096 rows -> 512
rows/core). Weights are loaded as raw int8 (HWDGE, Scalar ring) and
dequantized to fp16 on the Activation and Vector engines (int8 is exact
in fp16), pipelined so weight k-tile i lands just before the PE consumes
it. The x shard is cast to fp16 and transposed with one batched DMA-xbar
transpose per m-tile so K lands on partitions. The matmul loop is
k-outer across all 8 PSUM banks, and the per-channel scale is fused into
the PSUM->SBUF drain on the vector engine. Output DMAs ride the Scalar
HWDGE queue; x loads and transposes ride the Sync queue.
"""

import numpy as np

P = 128
B, S, D, F = 2, 2048, 1024, 4096
N_CORES = 8
M_FULL = B * S                    # 4096 rows
M_CORE = M_FULL // N_CORES        # 512 rows per core
NT = 512                          # n-tile (one PSUM bank of f32)
WM, WK, WN = M_CORE // P, D // P, F // NT
ACT_TILES = {1, 4, 7}             # weight k-tiles dequantized on ACT; rest on DVE

_CACHE: dict = {}


def _build():
    import concourse.tile as tile
    from concourse import bacc, mybir

    nc = bacc.Bacc("TRN2", target_bir_lowering=False, debug=False)

    x_dram = nc.dram_tensor("x", [M_CORE, D], mybir.dt.float32, kind="ExternalInput")
    w_dram = nc.dram_tensor("w", [D, F], mybir.dt.int8, kind="ExternalInput")
    s_dram = nc.dram_tensor("s", [1, F], mybir.dt.float32, kind="ExternalInput")
    o_dram = nc.dram_tensor("o", [M_CORE, F], mybir.dt.float32, kind="ExternalOutput")

    with tile.TileContext(nc) as tc:
        with (
            tc.tile_pool(name="wi", bufs=1) as wip,
            tc.tile_pool(name="w", bufs=1) as wp,
            tc.tile_pool(name="qs", bufs=1) as qp,
            tc.tile_pool(name="xf", bufs=2) as xfp,
            tc.tile_pool(name="xb", bufs=2) as xbp,
            tc.tile_pool(name="xt", bufs=2) as xtp,
            tc.tile_pool(name="o", bufs=8) as op,
            tc.tile_pool(name="ps", bufs=8, space="PSUM") as pp,
        ):
            # Raw int8 weight loads on the Scalar HWDGE ring.
            w_i8 = [
                wip.tile([P, F], mybir.dt.int8, name=f"wi{kt}", tag=f"wi{kt}")
                for kt in range(WK)
            ]
            for kt in range(WK):
                nc.scalar.dma_start(w_i8[kt][:], w_dram[kt * P:(kt + 1) * P, :])

            # First x tile load early on the Sync ring.
            xf0 = xfp.tile([P, D], mybir.dt.float32, name="xf0", tag="xf")
            nc.sync.dma_start(xf0[:], x_dram[0:P, :])

            # Dequant int8 -> fp16, split across ACT and DVE in k order.
            w_sb = [
                wp.tile([P, F], mybir.dt.float16, name=f"w{kt}", tag=f"w{kt}")
                for kt in range(WK)
            ]
            xb0 = xbp.tile([P, D], mybir.dt.float16, name="xb0", tag="xb")
            nc.vector.tensor_copy(xb0[:], xf0[:])
            for kt in range(WK):
                if kt in ACT_TILES:
                    nc.scalar.copy(w_sb[kt][:], w_i8[kt][:])
                else:
                    nc.vector.tensor_copy(w_sb[kt][:], w_i8[kt][:])

            # Per-channel scale replicated across partitions (HWDGE on Scalar,
            # queued behind the weight loads, ahead of the output stores).
            qs = qp.tile([P, F], mybir.dt.float32)
            nc.scalar.dma_start(qs[:], s_dram[0:1, :].to_broadcast((P, F)))

            xbs = [xb0]
            for mt in range(1, WM):
                xf = xfp.tile([P, D], mybir.dt.float32, name=f"xf{mt}", tag="xf")
                nc.sync.dma_start(xf[:], x_dram[mt * P:(mt + 1) * P, :])
                xb = xbp.tile([P, D], mybir.dt.float16, name=f"xb{mt}", tag="xb")
                nc.vector.tensor_copy(xb[:], xf[:])
                xbs.append(xb)

            for mt in range(WM):
                # One xbar transpose per m-tile: [128m,1024k] -> [128k,8kt,128m]
                xt = xtp.tile([P, WK, P], mybir.dt.float16, name=f"xt{mt}", tag="xt")
                nc.sync.dma_start(xt[:, :, :], xbs[mt][:, :], transpose=True)

                ps = [
                    pp.tile([P, NT], mybir.dt.float32, name=f"ps{mt}_{nt}", tag="ps")
                    for nt in range(WN)
                ]
                for kt in range(WK):
                    for nt in range(WN):
                        nc.tensor.matmul(
                            ps[nt][:],
                            xt[:, kt, :],
                            w_sb[kt][:, nt * NT:(nt + 1) * NT],
                            start=(kt == 0),
                            stop=(kt == WK - 1),
                        )
                for nt in range(WN):
                    ot = op.tile([P, NT], mybir.dt.float32, name=f"o{mt}_{nt}", tag="o")
                    nc.vector.tensor_mul(ot[:], ps[nt][:], qs[:, nt * NT:(nt + 1) * NT])
                    nc.scalar.dma_start(
                        o_dram[mt * P:(mt + 1) * P, nt * NT:(nt + 1) * NT], ot[:]
                    )

    nc.compile()
    return nc


def _get_nc():
    if "nc" not in _CACHE:
        _CACHE["nc"] = _build()
    return _CACHE["nc"]


def _run(x, qkernel, qscale, trace=False):
    from concourse.bass_utils import run_bass_kernel_spmd

    x = np.ascontiguousarray(np.asarray(x), dtype=np.float32).reshape(M_FULL, D)
    w = np.asarray(qkernel)
    if w.dtype != np.int8:
        w = w.astype(np.int8)
    w = np.ascontiguousarray(w)
    s = np.ascontiguousarray(np.asarray(qscale), dtype=np.float32).reshape(1, F)

    in_maps = [
        {"x": x[c * M_CORE:(c + 1) * M_CORE], "w": w, "s": s} for c in range(N_CORES)
    ]
    res = run_bass_kernel_spmd(
        _get_nc(), in_maps, core_ids=list(range(N_CORES)), trace=trace
    )
    out = np.concatenate([res.results[c]["o"] for c in range(N_CORES)], axis=0)
    return out.reshape(B, S, F), res


def kernel(x, qkernel, qscale):
    out, _ = _run(x, qkernel, qscale, trace=False)
    return out


def kernel_traced(x, qkernel, qscale):
    out, res = _run(x, qkernel, qscale, trace=True)
    return out, res


# revision 7
# speedup vs baseline: 1.4825x; 1.1864x over previous
"""DenseGeneralAqt inference kernel for Trainium2 (8 NeuronCores).

out = (x @ dequant_int8(qkernel)) * qscale,  x:(2,2048,1024) f32,
qkernel:(1024,4096) int8, qscale:(1,4096) f32 -> out:(2,2048,4096) f32.

Strategy: data-parallel over the flattened token axis (4096 rows -> 512
rows/core). Input marshalling transposes x to [D, M] so the contraction
dim lands on SBUF partitions (the same host pass that shards it). On
device, each core loads its xT shard + the full int8 weight, dequantizes
weight k-tiles to fp16 on the Activation and Vector engines (int8 is
exact in fp16) pipelined so k-tile i lands just before the PE consumes
it, casts x to fp16 on the vector engine, runs a k-outer matmul sweep
across all 8 PSUM banks per m-tile, and fuses the per-channel scale into
the PSUM->SBUF drain on the vector engine. Weight/scale/output DMAs ride
the Scalar HWDGE ring; x loads ride the Sync ring.
"""

import numpy as np

P = 128
B, S, D, F = 2, 2048, 1024, 4096
N_CORES = 8
M_FULL = B * S                    # 4096 rows
M_CORE = M_FULL // N_CORES        # 512 rows per core
NT = 512                          # n-tile (one PSUM bank of f32)
WM, WK, WN = M_CORE // P, D // P, F // NT
ACT_TILES = {1, 4, 7}             # weight k-tiles dequantized on ACT; rest on DVE
XDMA = 4                          # split the xT load into this many DMAs

_CACHE: dict = {}


def _build():
    import concourse.tile as tile
    from concourse import bacc, mybir

    nc = bacc.Bacc("TRN2", target_bir_lowering=False, debug=False)

    xt_dram = nc.dram_tensor("xt", [D, M_CORE], mybir.dt.float32, kind="ExternalInput")
    w_dram = nc.dram_tensor("w", [D, F], mybir.dt.int8, kind="ExternalInput")
    s_dram = nc.dram_tensor("s", [1, F], mybir.dt.float32, kind="ExternalInput")
    o_dram = nc.dram_tensor("o", [M_CORE, F], mybir.dt.float32, kind="ExternalOutput")

    xt_view = xt_dram[:, :].rearrange("(kt kp) m -> kp kt m", kp=P)  # [128, 8, 512]

    with tile.TileContext(nc) as tc:
        with (
            tc.tile_pool(name="wi", bufs=1) as wip,
            tc.tile_pool(name="w", bufs=1) as wp,
            tc.tile_pool(name="qs", bufs=1) as qp,
            tc.tile_pool(name="xf", bufs=1) as xfp,
            tc.tile_pool(name="xh", bufs=1) as xhp,
            tc.tile_pool(name="o", bufs=8) as op,
            tc.tile_pool(name="ps", bufs=8, space="PSUM") as pp,
        ):
            # Weight k-tile 0 first (gates the first matmul), then x, then rest.
            w_i8 = [
                wip.tile([P, F], mybir.dt.int8, name=f"wi{kt}", tag=f"wi{kt}")
                for kt in range(WK)
            ]
            nc.scalar.dma_start(w_i8[0][:], w_dram[0:P, :])

            # xT shard [128kp, 8kt, 512m] f32, split into a few DMAs (Sync ring).
            xf = xfp.tile([P, WK, M_CORE], mybir.dt.float32, name="xf", tag="xf")
            kper = WK // XDMA
            for i in range(XDMA):
                nc.sync.dma_start(
                    xf[:, i * kper:(i + 1) * kper, :],
                    xt_view[:, i * kper:(i + 1) * kper, :],
                )

            for kt in range(1, WK):
                nc.scalar.dma_start(w_i8[kt][:], w_dram[kt * P:(kt + 1) * P, :])

            # Cast x to fp16 per k-tile (DVE), in consumption order.
            xh = xhp.tile([P, WK, M_CORE], mybir.dt.float16, name="xh", tag="xh")
            for kt in range(WK):
                nc.vector.tensor_copy(xh[:, kt, :], xf[:, kt, :])

            # Dequant weights int8 -> fp16, split across ACT and DVE in k order.
            w_sb = [
                wp.tile([P, F], mybir.dt.float16, name=f"w{kt}", tag=f"w{kt}")
                for kt in range(WK)
            ]
            for kt in range(WK):
                if kt in ACT_TILES:
                    nc.scalar.copy(w_sb[kt][:], w_i8[kt][:])
                else:
                    nc.vector.tensor_copy(w_sb[kt][:], w_i8[kt][:])

            # Per-channel scale replicated across partitions (Scalar ring,
            # behind the weight loads, ahead of the output stores).
            qs = qp.tile([P, F], mybir.dt.float32)
            nc.scalar.dma_start(qs[:], s_dram[0:1, :].to_broadcast((P, F)))

            for mt in range(WM):
                ps = [
                    pp.tile([P, NT], mybir.dt.float32, name=f"ps{mt}_{nt}", tag="ps")
                    for nt in range(WN)
                ]
                for kt in range(WK):
                    for nt in range(WN):
                        nc.tensor.matmul(
                            ps[nt][:],
                            xh[:, kt, mt * P:(mt + 1) * P],
                            w_sb[kt][:, nt * NT:(nt + 1) * NT],
                            start=(kt == 0),
                            stop=(kt == WK - 1),
                        )
                for nt in range(WN):
                    ot = op.tile([P, NT], mybir.dt.float32, name=f"o{mt}_{nt}", tag="o")
                    nc.vector.tensor_mul(ot[:], ps[nt][:], qs[:, nt * NT:(nt + 1) * NT])
                    nc.scalar.dma_start(
                        o_dram[mt * P:(mt + 1) * P, nt * NT:(nt + 1) * NT], ot[:]
                    )

    nc.compile()
    return nc


def _get_nc():
    if "nc" not in _CACHE:
        _CACHE["nc"] = _build()
    return _CACHE["nc"]


def _run(x, qkernel, qscale, trace=False):
    from concourse.bass_utils import run_bass_kernel_spmd

    x = np.asarray(x, dtype=np.float32).reshape(M_FULL, D)
    xt = np.ascontiguousarray(x.T)                       # [D, M_FULL]
    w = np.asarray(qkernel)
    if w.dtype != np.int8:
        w = w.astype(np.int8)
    w = np.ascontiguousarray(w)
    s = np.ascontiguousarray(np.asarray(qscale), dtype=np.float32).reshape(1, F)

    in_maps = [
        {"xt": np.ascontiguousarray(xt[:, c * M_CORE:(c + 1) * M_CORE]), "w": w, "s": s}
        for c in range(N_CORES)
    ]
    res = run_bass_kernel_spmd(
        _get_nc(), in_maps, core_ids=list(range(N_CORES)), trace=trace
    )
    out = np.concatenate([res.results[c]["o"] for c in range(N_CORES)], axis=0)
    return out.reshape(B, S, F), res


def kernel(x, qkernel, qscale):
    out, _ = _run(x, qkernel, qscale, trace=False)
    return out


def kernel_traced(x, qkernel, qscale):
    out, res = _run(x, qkernel, qscale, trace=True)
    return out, res


# revision 8
# speedup vs baseline: 1.5003x; 1.0120x over previous
"""DenseGeneralAqt inference kernel for Trainium2 (8 NeuronCores).

out = (x @ dequant_int8(qkernel)) * qscale,  x:(2,2048,1024) f32,
qkernel:(1024,4096) int8, qscale:(1,4096) f32 -> out:(2,2048,4096) f32.

Strategy: data-parallel over the flattened token axis (4096 rows -> 512
rows/core). Input marshalling transposes x to [D, M] (contraction on
SBUF partitions) and casts it to fp16 — the same host pass that shards
it. On device, each core loads its xT shard (Sync ring) and the full
int8 weight (GPSIMD/SWDGE ring, k-tile 0 first), dequantizes each weight
k-tile to fp16 split column-wise across the Vector and Activation
engines (int8 is exact in fp16; the 2560/1536 split matches their
relative rates so combined convert throughput outruns PE consumption),
runs a k-outer matmul sweep across all 8 PSUM banks per m-tile, and
fuses the per-channel scale into the PSUM->SBUF drain on the vector
engine. Scale broadcast and output stores ride the Scalar HWDGE ring.
"""

import numpy as np

P = 128
B, S, D, F = 2, 2048, 1024, 4096
N_CORES = 8
M_FULL = B * S                    # 4096 rows
M_CORE = M_FULL // N_CORES        # 512 rows per core
NT = 512                          # n-tile (one PSUM bank of f32)
WM, WK, WN = M_CORE // P, D // P, F // NT
CSPLIT = 5 * NT                   # weight dequant: DVE cols [0:2560), ACT [2560:4096)

_CACHE: dict = {}


def _build():
    import concourse.tile as tile
    from concourse import bacc, mybir

    nc = bacc.Bacc("TRN2", target_bir_lowering=False, debug=False)

    xt_dram = nc.dram_tensor("xt", [D, M_CORE], mybir.dt.float16, kind="ExternalInput")
    w_dram = nc.dram_tensor("w", [D, F], mybir.dt.int8, kind="ExternalInput")
    s_dram = nc.dram_tensor("s", [1, F], mybir.dt.float32, kind="ExternalInput")
    o_dram = nc.dram_tensor("o", [M_CORE, F], mybir.dt.float32, kind="ExternalOutput")

    xt_view = xt_dram[:, :].rearrange("(kt kp) m -> kp kt m", kp=P)  # [128, 8, 512]

    with tile.TileContext(nc) as tc:
        with (
            tc.tile_pool(name="wi", bufs=1) as wip,
            tc.tile_pool(name="w", bufs=1) as wp,
            tc.tile_pool(name="qs", bufs=1) as qp,
            tc.tile_pool(name="xh", bufs=1) as xhp,
            tc.tile_pool(name="o", bufs=8) as op,
            tc.tile_pool(name="ps", bufs=8, space="PSUM") as pp,
        ):
            # Weight loads on the GPSIMD (SWDGE) ring; k-tile 0 first since it
            # gates the first matmul and the first dequant.
            w_i8 = [
                wip.tile([P, F], mybir.dt.int8, name=f"wi{kt}", tag=f"wi{kt}")
                for kt in range(WK)
            ]
            nc.gpsimd.dma_start(w_i8[0][:], w_dram[0:P, :])

            # xT shard [128kp, 8kt, 512m] fp16 in two DMAs on the Sync ring.
            xh = xhp.tile([P, WK, M_CORE], mybir.dt.float16, name="xh", tag="xh")
            half = WK // 2
            nc.sync.dma_start(xh[:, 0:half, :], xt_view[:, 0:half, :])
            nc.sync.dma_start(xh[:, half:WK, :], xt_view[:, half:WK, :])

            for kt in range(1, WK):
                nc.gpsimd.dma_start(w_i8[kt][:], w_dram[kt * P:(kt + 1) * P, :])

            # Dequant int8 -> fp16, each k-tile split DVE | ACT by column.
            w_sb = [
                wp.tile([P, F], mybir.dt.float16, name=f"w{kt}", tag=f"w{kt}")
                for kt in range(WK)
            ]
            qs = qp.tile([P, F], mybir.dt.float32)
            for kt in range(WK):
                nc.vector.tensor_copy(w_sb[kt][:, 0:CSPLIT], w_i8[kt][:, 0:CSPLIT])
                nc.scalar.copy(w_sb[kt][:, CSPLIT:F], w_i8[kt][:, CSPLIT:F])
                if kt == 0:
                    # Scale broadcast on the Scalar ring after the first dequant
                    # chunk, well before the first PSUM drain needs it.
                    nc.scalar.dma_start(qs[:], s_dram[0:1, :].to_broadcast((P, F)))

            for mt in range(WM):
                ps = [
                    pp.tile([P, NT], mybir.dt.float32, name=f"ps{mt}_{nt}", tag="ps")
                    for nt in range(WN)
                ]
                for kt in range(WK):
                    for nt in range(WN):
                        nc.tensor.matmul(
                            ps[nt][:],
                            xh[:, kt, mt * P:(mt + 1) * P],
                            w_sb[kt][:, nt * NT:(nt + 1) * NT],
                            start=(kt == 0),
                            stop=(kt == WK - 1),
                        )
                for nt in range(WN):
                    ot = op.tile([P, NT], mybir.dt.float32, name=f"o{mt}_{nt}", tag="o")
                    nc.vector.tensor_mul(ot[:], ps[nt][:], qs[:, nt * NT:(nt + 1) * NT])
                    nc.scalar.dma_start(
                        o_dram[mt * P:(mt + 1) * P, nt * NT:(nt + 1) * NT], ot[:]
                    )

    nc.compile()
    return nc


def _get_nc():
    if "nc" not in _CACHE:
        _CACHE["nc"] = _build()
    return _CACHE["nc"]


def _run(x, qkernel, qscale, trace=False):
    from concourse.bass_utils import run_bass_kernel_spmd

    x = np.asarray(x, dtype=np.float32).reshape(M_FULL, D)
    xt = np.ascontiguousarray(x.T).astype(np.float16)    # [D, M_FULL]
    w = np.asarray(qkernel)
    if w.dtype != np.int8:
        w = w.astype(np.int8)
    w = np.ascontiguousarray(w)
    s = np.ascontiguousarray(np.asarray(qscale), dtype=np.float32).reshape(1, F)

    in_maps = [
        {"xt": np.ascontiguousarray(xt[:, c * M_CORE:(c + 1) * M_CORE]), "w": w, "s": s}
        for c in range(N_CORES)
    ]
    res = run_bass_kernel_spmd(
        _get_nc(), in_maps, core_ids=list(range(N_CORES)), trace=trace
    )
    out = np.concatenate([res.results[c]["o"] for c in range(N_CORES)], axis=0)
    return out.reshape(B, S, F), res


def kernel(x, qkernel, qscale):
    out, _ = _run(x, qkernel, qscale, trace=False)
    return out


def kernel_traced(x, qkernel, qscale):
    out, res = _run(x, qkernel, qscale, trace=True)
    return out, res


# revision 11
# speedup vs baseline: 1.5334x; 1.0221x over previous
"""DenseGeneralAqt inference kernel for Trainium2 (8 NeuronCores).

out = (x @ dequant_int8(qkernel)) * qscale,  x:(2,2048,1024) f32,
qkernel:(1024,4096) int8, qscale:(1,4096) f32 -> out:(2,2048,4096) f32.

Strategy: data-parallel over the flattened token axis (4096 rows -> 512
rows/core). Input marshalling transposes x to [D, M] (contraction on
SBUF partitions) and casts it to fp16 — the same host pass that shards
it. On device, each core loads its xT shard (Sync ring) and the full
int8 weight (GPSIMD/SWDGE ring, k-tile 0 first), dequantizes each weight
k-tile to fp16 split column-wise across the Vector and Activation
engines (int8 is exact in fp16; the 2560/1536 split matches their
relative rates so combined convert throughput outruns PE consumption),
runs a k-outer matmul sweep across all 8 PSUM banks per m-tile, and
fuses the per-channel scale into the PSUM->SBUF drain on the vector
engine. Scale broadcast and output stores ride the Scalar HWDGE ring.
"""

import numpy as np

P = 128
B, S, D, F = 2, 2048, 1024, 4096
N_CORES = 8
M_FULL = B * S                    # 4096 rows
M_CORE = M_FULL // N_CORES        # 512 rows per core
NT = 512                          # n-tile (one PSUM bank of f32)
WM, WK, WN = M_CORE // P, D // P, F // NT
CSPLIT = 5 * NT                   # weight dequant: DVE cols [0:2560), ACT [2560:4096)

_CACHE: dict = {}


def _build():
    import concourse.tile as tile
    from concourse import bacc, mybir

    nc = bacc.Bacc("TRN2", target_bir_lowering=False, debug=False)

    xt_dram = nc.dram_tensor("xt", [D, M_CORE], mybir.dt.float16, kind="ExternalInput")
    w_dram = nc.dram_tensor("w", [D, F], mybir.dt.int8, kind="ExternalInput")
    s_dram = nc.dram_tensor("s", [1, F], mybir.dt.float32, kind="ExternalInput")
    o_dram = nc.dram_tensor("o", [M_CORE, F], mybir.dt.float32, kind="ExternalOutput")

    xt_view = xt_dram[:, :].rearrange("(kt kp) m -> kp kt m", kp=P)  # [128, 8, 512]

    with tile.TileContext(nc) as tc:
        with (
            tc.tile_pool(name="wi", bufs=1) as wip,
            tc.tile_pool(name="w", bufs=1) as wp,
            tc.tile_pool(name="qs", bufs=1) as qp,
            tc.tile_pool(name="xh", bufs=1) as xhp,
            tc.tile_pool(name="o", bufs=8) as op,
            tc.tile_pool(name="ps", bufs=8, space="PSUM") as pp,
        ):
            # Weight loads on the GPSIMD (SWDGE) ring; k-tile 0 first since it
            # gates the first matmul and the first dequant.
            w_i8 = [
                wip.tile([P, F], mybir.dt.int8, name=f"wi{kt}", tag=f"wi{kt}")
                for kt in range(WK)
            ]
            nc.gpsimd.dma_start(w_i8[0][:], w_dram[0:P, :])

            # xT shard [128kp, 8kt, 512m] fp16 in two DMAs on the Sync ring.
            xh = xhp.tile([P, WK, M_CORE], mybir.dt.float16, name="xh", tag="xh")
            half = WK // 2
            nc.sync.dma_start(xh[:, 0:half, :], xt_view[:, 0:half, :])
            nc.sync.dma_start(xh[:, half:WK, :], xt_view[:, half:WK, :])

            for kt in range(1, WK):
                nc.gpsimd.dma_start(w_i8[kt][:], w_dram[kt * P:(kt + 1) * P, :])

            # Dequant int8 -> fp16, each k-tile split DVE | ACT by column.
            w_sb = [
                wp.tile([P, F], mybir.dt.float16, name=f"w{kt}", tag=f"w{kt}")
                for kt in range(WK)
            ]
            qs = qp.tile([P, F], mybir.dt.float32)
            cv_dve = []
            for kt in range(WK):
                cv_dve.append(
                    nc.vector.tensor_copy(w_sb[kt][:, 0:CSPLIT], w_i8[kt][:, 0:CSPLIT])
                )
                nc.scalar.copy(w_sb[kt][:, CSPLIT:F], w_i8[kt][:, CSPLIT:F])
            # Scale broadcast (2 MB DRE replication): hold it back until the
            # kt=1 dequant so its bytes don't starve the critical w0/x loads;
            # still lands well before the first PSUM drain needs it.
            qs_dma = nc.scalar.dma_start(qs[:], s_dram[0:1, :].to_broadcast((P, F)))
            tile.add_dep_helper(qs_dma.ins, cv_dve[1].ins, reason="defer qs broadcast")

            for mt in range(WM):
                ps = [
                    pp.tile([P, NT], mybir.dt.float32, name=f"ps{mt}_{nt}", tag="ps")
                    for nt in range(WN)
                ]

                def drain(mt, nt):
                    ot = op.tile([P, NT], mybir.dt.float32, name=f"o{mt}_{nt}", tag="o")
                    nc.vector.tensor_mul(ot[:], ps[nt][:], qs[:, nt * NT:(nt + 1) * NT])
                    nc.scalar.dma_start(
                        o_dram[mt * P:(mt + 1) * P, nt * NT:(nt + 1) * NT], ot[:]
                    )

                if mt < WM - 1:
                    # k-outer: consume each weight k-tile across all 8 banks as
                    # soon as it is dequantized (hides the dequant pipeline).
                    for kt in range(WK):
                        for nt in range(WN):
                            nc.tensor.matmul(
                                ps[nt][:],
                                xh[:, kt, mt * P:(mt + 1) * P],
                                w_sb[kt][:, nt * NT:(nt + 1) * NT],
                                start=(kt == 0),
                                stop=(kt == WK - 1),
                            )
                    for nt in range(WN):
                        drain(mt, nt)
                else:
                    # Last m-tile: n-outer so each bank's reduction finishes
                    # early and the tail drains overlap the remaining matmuls.
                    for nt in range(WN):
                        for kt in range(WK):
                            nc.tensor.matmul(
                                ps[nt][:],
                                xh[:, kt, mt * P:(mt + 1) * P],
                                w_sb[kt][:, nt * NT:(nt + 1) * NT],
                                start=(kt == 0),
                                stop=(kt == WK - 1),
                            )
                        drain(mt, nt)

    nc.compile()
    return nc


def _get_nc():
    if "nc" not in _CACHE:
        _CACHE["nc"] = _build()
    return _CACHE["nc"]


def _run(x, qkernel, qscale, trace=False):
    from concourse.bass_utils import run_bass_kernel_spmd

    x = np.asarray(x, dtype=np.float32).reshape(M_FULL, D)
    xt = np.ascontiguousarray(x.T).astype(np.float16)    # [D, M_FULL]
    w = np.asarray(qkernel)
    if w.dtype != np.int8:
        w = w.astype(np.int8)
    w = np.ascontiguousarray(w)
    s = np.ascontiguousarray(np.asarray(qscale), dtype=np.float32).reshape(1, F)

    in_maps = [
        {"xt": np.ascontiguousarray(xt[:, c * M_CORE:(c + 1) * M_CORE]), "w": w, "s": s}
        for c in range(N_CORES)
    ]
    res = run_bass_kernel_spmd(
        _get_nc(), in_maps, core_ids=list(range(N_CORES)), trace=trace
    )
    out = np.concatenate([res.results[c]["o"] for c in range(N_CORES)], axis=0)
    return out.reshape(B, S, F), res


def kernel(x, qkernel, qscale):
    out, _ = _run(x, qkernel, qscale, trace=False)
    return out


def kernel_traced(x, qkernel, qscale):
    out, res = _run(x, qkernel, qscale, trace=True)
    return out, res


# revision 12
# speedup vs baseline: 1.7015x; 1.1096x over previous
"""DenseGeneralAqt inference kernel for Trainium2 (8 NeuronCores).

out = (x @ dequant_int8(qkernel)) * qscale,  x:(2,2048,1024) f32,
qkernel:(1024,4096) int8, qscale:(1,4096) f32 -> out:(2,2048,4096) f32.

Strategy: 2D sharding — 4-way over the flattened token axis (M) x 2-way
over features (N). That minimizes per-core input traffic (2.1 MB x +
2.1 MB w, the HBM-bound startup phase). Input marshalling transposes x
to [D, M] (contraction on SBUF partitions) and casts it to fp16, the
same host pass that shards it. On device each core loads its xT shard
(Sync ring) and its int8 weight half (GPSIMD/SWDGE ring, k-tile 0
first), dequantizes weight k-tiles to fp16 on the vector engine (int8 is
exact in fp16, pipelined ahead of PE consumption), then sweeps m-tile
pairs k-outer across all 8 PSUM banks; the per-channel scale (replicated
across partitions by a deferred DRE-broadcast DMA) is fused into the
PSUM->SBUF drain on the vector engine. The last sweep runs n-outer so
its drains overlap the remaining matmuls. Output stores ride the Scalar
HWDGE ring.
"""

import numpy as np

P = 128
B, S, D, F = 2, 2048, 1024, 4096
N_CORES = 8
MSH, NSH = 4, 2                   # shard grid: 4 m-blocks x 2 n-blocks
M_FULL = B * S                    # 4096 rows
M_CORE = M_FULL // MSH            # 1024 rows per core
N_CORE = F // NSH                 # 2048 cols per core
NT = 512                          # n-tile (one PSUM bank of f32)
WM, WK, WN = M_CORE // P, D // P, N_CORE // NT
XDMA = 4                          # xT load split (2 k-tiles per DMA)

_CACHE: dict = {}


def _build():
    import concourse.tile as tile
    from concourse import bacc, mybir

    nc = bacc.Bacc("TRN2", target_bir_lowering=False, debug=False)

    xt_dram = nc.dram_tensor("xt", [D, M_CORE], mybir.dt.float16, kind="ExternalInput")
    w_dram = nc.dram_tensor("w", [D, N_CORE], mybir.dt.int8, kind="ExternalInput")
    s_dram = nc.dram_tensor("s", [1, N_CORE], mybir.dt.float32, kind="ExternalInput")
    o_dram = nc.dram_tensor("o", [M_CORE, N_CORE], mybir.dt.float32, kind="ExternalOutput")

    xt_view = xt_dram[:, :].rearrange("(kt kp) m -> kp kt m", kp=P)  # [128, 8, 1024]

    with tile.TileContext(nc) as tc:
        with (
            tc.tile_pool(name="wi", bufs=1) as wip,
            tc.tile_pool(name="w", bufs=1) as wp,
            tc.tile_pool(name="qs", bufs=1) as qp,
            tc.tile_pool(name="xh", bufs=1) as xhp,
            tc.tile_pool(name="o", bufs=10) as op,
            tc.tile_pool(name="ps", bufs=8, space="PSUM") as pp,
        ):
            # Weight k-tile 0 first (gates the first dequant + matmul).
            w_i8 = [
                wip.tile([P, N_CORE], mybir.dt.int8, name=f"wi{kt}", tag=f"wi{kt}")
                for kt in range(WK)
            ]
            nc.gpsimd.dma_start(w_i8[0][:], w_dram[0:P, :])

            # xT shard [128kp, 8kt, 1024m] fp16 on the Sync ring.
            xh = xhp.tile([P, WK, M_CORE], mybir.dt.float16, name="xh", tag="xh")
            kper = WK // XDMA
            for i in range(XDMA):
                nc.sync.dma_start(
                    xh[:, i * kper:(i + 1) * kper, :],
                    xt_view[:, i * kper:(i + 1) * kper, :],
                )

            for kt in range(1, WK):
                nc.gpsimd.dma_start(w_i8[kt][:], w_dram[kt * P:(kt + 1) * P, :])

            # Dequant int8 -> fp16 on the vector engine, in k order.
            w_sb = [
                wp.tile([P, N_CORE], mybir.dt.float16, name=f"w{kt}", tag=f"w{kt}")
                for kt in range(WK)
            ]
            cv = [nc.vector.tensor_copy(w_sb[kt][:], w_i8[kt][:]) for kt in range(WK)]

            # Scale broadcast (1 MB DRE replication): deferred so its bytes
            # don't starve the critical early loads; lands before first drain.
            qs = qp.tile([P, N_CORE], mybir.dt.float32)
            qs_dma = nc.scalar.dma_start(qs[:], s_dram[0:1, :].to_broadcast((P, N_CORE)))
            tile.add_dep_helper(qs_dma.ins, cv[2].ins, reason="defer qs broadcast")

            def drain(mi, nt, ps_tile):
                ot = op.tile([P, NT], mybir.dt.float32, name=f"o{mi}_{nt}", tag="o")
                nc.vector.tensor_mul(ot[:], ps_tile[:], qs[:, nt * NT:(nt + 1) * NT])
                nc.scalar.dma_start(
                    o_dram[mi * P:(mi + 1) * P, nt * NT:(nt + 1) * NT], ot[:]
                )

            def mm(ps_tile, kt, mi, nt, first, last):
                nc.tensor.matmul(
                    ps_tile[:],
                    xh[:, kt, mi * P:(mi + 1) * P],
                    w_sb[kt][:, nt * NT:(nt + 1) * NT],
                    start=first,
                    stop=last,
                )

            # m-tile pairs x 4 n-tiles = 8 PSUM banks per k-outer sweep.
            pairs = [(2 * i, 2 * i + 1) for i in range(WM // 2)]
            for pi, pair in enumerate(pairs):
                combos = [(mi, nt) for mi in pair for nt in range(WN)]
                if pi < len(pairs) - 1:
                    # k-outer: consume each weight k-tile across all 8 banks
                    # as soon as it is dequantized.
                    ps = {
                        c: pp.tile([P, NT], mybir.dt.float32, name=f"ps{pi}_{c[0]}_{c[1]}", tag="ps")
                        for c in combos
                    }
                    for kt in range(WK):
                        for c in combos:
                            mm(ps[c], kt, c[0], c[1], kt == 0, kt == WK - 1)
                    for c in combos:
                        drain(c[0], c[1], ps[c])
                else:
                    # Last sweep: n-outer so each bank's reduction finishes
                    # early and the tail drains overlap the remaining matmuls.
                    for c in combos:
                        ps_t = pp.tile([P, NT], mybir.dt.float32, name=f"ps{pi}_{c[0]}_{c[1]}", tag="ps")
                        for kt in range(WK):
                            mm(ps_t, kt, c[0], c[1], kt == 0, kt == WK - 1)
                        drain(c[0], c[1], ps_t)

    nc.compile()
    return nc


def _get_nc():
    if "nc" not in _CACHE:
        _CACHE["nc"] = _build()
    return _CACHE["nc"]


def _run(x, qkernel, qscale, trace=False):
    from concourse.bass_utils import run_bass_kernel_spmd

    x = np.asarray(x, dtype=np.float32).reshape(M_FULL, D)
    xt = np.ascontiguousarray(x.T).astype(np.float16)    # [D, M_FULL]
    w = np.asarray(qkernel)
    if w.dtype != np.int8:
        w = w.astype(np.int8)
    s = np.asarray(qscale, dtype=np.float32).reshape(1, F)

    in_maps = []
    for c in range(N_CORES):
        mb, nb = c % MSH, c // MSH
        in_maps.append({
            "xt": np.ascontiguousarray(xt[:, mb * M_CORE:(mb + 1) * M_CORE]),
            "w": np.ascontiguousarray(w[:, nb * N_CORE:(nb + 1) * N_CORE]),
            "s": np.ascontiguousarray(s[:, nb * N_CORE:(nb + 1) * N_CORE]),
        })
    res = run_bass_kernel_spmd(
        _get_nc(), in_maps, core_ids=list(range(N_CORES)), trace=trace
    )
    out = np.empty((M_FULL, F), dtype=np.float32)
    for c in range(N_CORES):
        mb, nb = c % MSH, c // MSH
        out[mb * M_CORE:(mb + 1) * M_CORE, nb * N_CORE:(nb + 1) * N_CORE] = res.results[c]["o"]
    return out.reshape(B, S, F), res


def kernel(x, qkernel, qscale):
    out, _ = _run(x, qkernel, qscale, trace=False)
    return out


def kernel_traced(x, qkernel, qscale):
    out, res = _run(x, qkernel, qscale, trace=True)
    return out, res
